# revision 4
# baseline (speedup 1.0000x reference)
"""Trainium2 Bass kernel for nn_FusedKQnA (sparse attention with learned
queries + depthwise stride-2 conv aggregation).

Math restructuring (vs the reference):
  - k never materialized: qkT = x^T @ (Wk @ QW), QW block-diagonal learned
    queries -> one (128->32) matmul producing cost = exp(qkT).
  - Global max subtractions inside the exps cancel between num/den -> dropped.
  - r = 1/sum_den computed as exp(-ln(den)); den conv done as 9 diagonal
    matmuls on the PE (col-tiled 4-way over strip x batch).
  - gamma[t,h,o] = sum_q kern[t,qh] * r[qh,o] * cost[n_t(o),qh] via a one-hot
    PE matmul (3 taps row-tiled), out_pre = sum_t gamma_t * v_t elementwise.

Layout: all spatial maps are stored PHASE-DECOMPOSED (rows and cols split by
parity).  Every stride-2 tap view of the conv becomes a dense step-1 view of
one phase plane, so all DVE tensor_tensor ops run in 2x packed mode.  Even-
column phases store data at column offset 2 (odd offset 1 reserved for the
zero border) so reads/writes stay 4-byte aligned; the residual +-1 column
shifts between taps are absorbed by three shifted placements of r (r0/r1/r2)
and one offset add at the end.

Sharding: pure data parallel over batch: 16 batches -> 8 cores x 2.
"""

import os
from contextlib import ExitStack

import numpy as np

import concourse.bass as bass
import concourse.mybir as mybir
import concourse.tile as tile
from concourse import bacc
from concourse.bass_utils import run_bass_kernel_spmd

# Problem constants (hardcoded per spec nn_FusedKQnA_1726576854813)
N_Q, N_HEADS, KSIZE, STRIDE, PADDING = 4, 4, 3, 2, 1
B, C, H, W = 16, 128, 56, 56
HC = C // N_HEADS            # 32 head channels
HP = N_HEADS * STRIDE        # 8 effective heads
CS = C * STRIDE              # 256
G = N_Q * HP                 # 32 kernel groups
HO, WO = H // STRIDE, W // STRIDE   # 28, 28
NCORES = 8
BPC = B // NCORES            # batches per core

F32 = mybir.dt.float32
BF16 = mybir.dt.bfloat16

# Phase-plane geometry.  Padded input coordinate q = x + 1 (q in 0..57).
# Row phases: pr = q_r % 2.  Col phases: pc = q_c % 2, stored at position
#   pc==0 (even q_c):  pos = q_c/2 + 1      (data pos 2..29, border pos 1)
#   pc==1 (odd  q_c):  pos = (q_c-1)/2      (data pos 0..27)
# Tap (di,dj) reads phase (pr(di), pc(dj)) at row a+RO[di], col c+CO[dj].
PW = 32                      # plane width (padded, 64B rows)
PR = {-1: 0, 0: 1, 1: 0}     # row phase per di
RO = {-1: 0, 0: 0, 1: 1}     # row offset per di
PCC = {-1: 0, 0: 1, 1: 0}    # col phase per dj
CO = {-1: 1, 0: 0, 1: 2}     # col position offset per dj
# groups by dj (gamma/product groups); taus by di
DJS = [-1, 0, 1]
DIS = [-1, 0, 1]

_BUILD_CACHE = {}


def _host_weights(Wk, Wv, Wout, q_param, attn_scale, rpb_table):
    """Precompute all small weight tensors on the host."""
    import ml_dtypes
    q = q_param.reshape(N_Q, HP, HC).astype(np.float64) * (HC ** -0.5)
    QW = np.zeros((CS, G), np.float64)
    for qi in range(N_Q):
        for h in range(HP):
            QW[h * HC:(h + 1) * HC, qi * HP + h] = q[qi, h]
    wkq = (Wk.astype(np.float64) @ QW).astype(np.float32)        # (128, 32)

    rpb_exp = np.exp(rpb_table.astype(np.float64))               # (9, 32)
    kern_num = (rpb_exp * attn_scale.astype(np.float64))         # (9, 32)

    # den conv kernels as diagonal matmul weights, partition-major: (32,9,32)
    denk = np.zeros((G, KSIZE * KSIZE, G), np.float32)
    for t in range(KSIZE * KSIZE):
        for g in range(G):
            denk[g, t, g] = rpb_exp[t, g]

    # gamma broadcast stationary: kmat[tau*32+g, grp, chn, p]
    # tap t_ref = tau*3 + grp  (tau = di+1, grp = dj+1)
    kmat = np.zeros((3 * G, 3, 2, 128), np.float32)
    for tau in range(3):
        for grp in range(3):
            t = tau * 3 + grp
            for g in range(G):
                h = g % HP
                chn = h // 4
                m0 = (h % 4) * HC
                kmat[tau * G + g, grp, chn, m0:m0 + HC] = kern_num[t, g]

    # woutT[c, kc, m] = Wout[m, kc*128 + c]
    woutT = np.ascontiguousarray(
        Wout.T.astype(np.float32).reshape(2, 128, CS).transpose(1, 0, 2))

    cast = lambda a: np.ascontiguousarray(a.astype(ml_dtypes.bfloat16))
    return dict(wkq=cast(wkq), wv=cast(Wv.astype(np.float32)),
                denk=cast(denk), kmat=cast(kmat), woutT=cast(woutT))


def _build_program():
    nc = bacc.Bacc("TRN2", target_bir_lowering=False, debug=False,
                   enable_asserts=False, num_devices=NCORES)

    x_d = nc.dram_tensor("x", [BPC, C, H, W], BF16, kind="ExternalInput").ap()
    wkq_d = nc.dram_tensor("wkq", [C, G], BF16, kind="ExternalInput").ap()
    wv_d = nc.dram_tensor("wv", [C, CS], BF16, kind="ExternalInput").ap()
    denk_d = nc.dram_tensor("denk", [G, 9, G], BF16, kind="ExternalInput").ap()
    kmat_d = nc.dram_tensor("kmat", [3 * G, 3, 2, 128], BF16,
                            kind="ExternalInput").ap()
    woutT_d = nc.dram_tensor("woutT", [128, 2, CS], BF16,
                             kind="ExternalInput").ap()
    out_d = nc.dram_tensor("out", [BPC, CS, HO, WO], F32,
                           kind="ExternalOutput").ap()

    with tile.TileContext(nc) as tc, ExitStack() as ctx:
        _kernel_body(ctx, tc, out_d, x_d, wkq_d, wv_d, denk_d, kmat_d, woutT_d)

    _pin_act_tables()
    nc.compile()
    return nc


def _pin_act_tables():
    """Force one ACT table set (natural_log_exp_and_others) for Exp+Ln."""
    from concourse import hw_specs
    import concourse.bacc as bacc_mod
    if getattr(bacc_mod, "_act_tables_pinned", False):
        return
    orig = hw_specs.get_activation_tables

    def patched(arch):
        tabs = dict(orig(arch))
        keep = "natural_log_exp_and_others"
        for name in list(tabs):
            if name == keep:
                continue
            fns = tabs[name]
            if any(str(f).endswith((".Exp", ".Ln")) for f in fns):
                tabs[name] = type(fns)()
        return tabs

    bacc_mod.get_activation_tables = patched
    bacc_mod._act_tables_pinned = True


def _kernel_body(ctx, tc, out_d, x_d, wkq_d, wv_d, denk_d, kmat_d, woutT_d):
    nc = tc.nc
    S = 14                   # rows per strip (2 strips of the 28 output rows)

    consts = ctx.enter_context(tc.tile_pool(name="consts", bufs=1))
    planes = ctx.enter_context(tc.tile_pool(name="planes", bufs=1))
    xpool = ctx.enter_context(tc.tile_pool(name="xpool", bufs=2))
    small = ctx.enter_context(tc.tile_pool(name="small", bufs=1))
    rcpool = ctx.enter_context(tc.tile_pool(name="rcpool", bufs=1))
    gampool = ctx.enter_context(tc.tile_pool(name="gampool", bufs=2))
    prod_pool = ctx.enter_context(tc.tile_pool(name="prod", bufs=3))
    spool = ctx.enter_context(tc.tile_pool(name="spool", bufs=2))
    opre_pool = ctx.enter_context(tc.tile_pool(name="opre", bufs=2))
    outs_pool = ctx.enter_context(tc.tile_pool(name="outs", bufs=2))

    ps = ctx.enter_context(tc.tile_pool(name="ps", bufs=2, space="PSUM"))

    # ---- constants into SBUF ----
    wkq_sb = consts.tile([C, G], BF16)
    nc.sync.dma_start(out=wkq_sb, in_=wkq_d)
    wv_sb = consts.tile([C, CS], BF16)
    nc.sync.dma_start(out=wv_sb, in_=wv_d)
    denk_sb = consts.tile([G, 9, G], BF16)
    nc.sync.dma_start(out=denk_sb, in_=denk_d)
    kmat_sb = consts.tile([3 * G, 3, 2, 128], BF16)
    nc.sync.dma_start(out=kmat_sb, in_=kmat_d)
    woutT_sb = consts.tile([128, 2, CS], BF16)
    nc.sync.dma_start(out=woutT_sb, in_=woutT_d)

    # ---- persistent phase planes; only borders/pads must be zeroed ----
    # plane index p = pr*2 + pc; shape [part, 4, 29, PW]
    cost_pl = [planes.tile([G, 4, 29, PW], BF16, tag=f"cost{b}",
                           name=f"cost_pl{b}") for b in range(BPC)]
    v_pl = [[planes.tile([128, 4, 29, PW], BF16, tag=f"v{b}_{chn}",
                         name=f"v_pl{b}_{chn}") for chn in range(2)]
            for b in range(BPC)]
    for pl in [cost_pl[b] for b in range(BPC)] + [v_pl[b][c] for b in range(BPC)
                                                  for c in range(2)]:
        # pr=0 planes (idx 0,1): row 0 is the top border
        nc.gpsimd.memset(pl[:, 0:2, 0, :], 0.0)
        # pc=0 planes (idx 0,2): cols 0..1 (left border at pos 1)
        nc.gpsimd.memset(pl[:, 0:4:2, :, 0:2], 0.0)
        # pc=1 planes (idx 1,3): cols 28..29 (right pad read by full-width ops)
        nc.gpsimd.memset(pl[:, 1:4:2, :, 28:30], 0.0)

    # r with three shifted placements: r_all[:, grp, a, CO[dj]+c]
    r_all = [planes.tile([G, 3, HO, PW], BF16, tag=f"r{b}", name=f"r_all{b}")
             for b in range(BPC)]
    for b in range(BPC):
        nc.gpsimd.memset(r_all[b][:, 0, :, 0:1], 0.0)    # grp0 (co=1): pos 0
        nc.gpsimd.memset(r_all[b][:, 0, :, 29:30], 0.0)
        nc.gpsimd.memset(r_all[b][:, 1, :, 28:30], 0.0)  # grp1 (co=0)
        nc.gpsimd.memset(r_all[b][:, 2, :, 0:2], 0.0)    # grp2 (co=2)

    # ---- load x ----
    x_sb = {}
    for b in range(BPC):
        x_sb[b] = xpool.tile([C, H, W], BF16, name=f"x_sb{b}")
        nc.sync.dma_start(out=x_sb[b], in_=x_d[b])

    def xview(b, pr, pc, s):
        rs = (1 if pr == 0 else 0) + 28 * s
        cs0 = 1 if pc == 0 else 0
        return x_sb[b][:, rs:min(rs + 28, 56):2, cs0:56:2]

    def plane_write(pl, pr, pc, s):
        p = pr * 2 + pc
        wr = (1 if pr == 0 else 0) + S * s
        wc = 2 if pc == 0 else 0
        return pl[:, p, wr:wr + S, wc:wc + 28]

    def tap_view(pl, di, dj, rows, width):
        p = PR[di] * 2 + PCC[dj]
        r0 = RO[di] + (0 if rows is None else S * rows)
        nr = 28 if rows is None else S
        return pl[:, p, r0:r0 + nr, 0:width]

    # ---- qkT: 4 phases col-tiled into one PSUM bank; exp -> cost planes ----
    for b in range(BPC):
        for s in range(2):
            qk_ps = ps.tile([128, 392], F32, tag="mm", bufs=2, name="qk_ps")
            for pr in range(2):
                for pc in range(2):
                    j = pr * 2 + pc
                    nc.tensor.matmul(qk_ps[32 * j:32 * (j + 1), :], wkq_sb,
                                     xview(b, pr, pc, s), start=True,
                                     stop=True, tile_position=(0, 32 * j))
            for pr in range(2):
                for pc in range(2):
                    j = pr * 2 + pc
                    nc.scalar.activation(
                        out=plane_write(cost_pl[b], pr, pc, s),
                        in_=qk_ps[32 * j:32 * (j + 1), :].rearrange(
                            "p (a c) -> p a c", a=S),
                        func=mybir.ActivationFunctionType.Exp)

    # ---- den: 9 diagonal matmuls, col-tiled 4-way over (b, s) ----
    den_ps = ps.tile([128, 3, 512], F32, tag="gam", bufs=2, name="den_ps")
    for t_i, (di, dj) in enumerate([(di, dj) for di in DIS for dj in DJS]):
        for b in range(BPC):
            for s in range(2):
                j = 2 * b + s
                dv = den_ps[32 * j:32 * (j + 1), 0, :392].rearrange(
                    "p (a c) -> p a c", a=S)
                nc.tensor.matmul(
                    dv, denk_sb[:, t_i, :],
                    tap_view(cost_pl[b], di, dj, s, None)[:, :, CO[dj]:CO[dj] + 28],
                    start=(t_i == 0), stop=(t_i == 8),
                    tile_position=(0, 32 * j))

    lden = small.tile([128, 392], F32, tag="lden", name="lden")
    nc.scalar.activation(out=lden, in_=den_ps[:, 0, :392],
                         func=mybir.ActivationFunctionType.Ln)
    for b in range(BPC):
        for gi in range(3):
            co = CO[DJS[gi]]
            for s in range(2):
                j = 2 * b + s
                nc.scalar.activation(
                    out=r_all[b][:, gi, S * s:S * (s + 1), co:co + 28],
                    in_=lden[32 * j:32 * (j + 1), :].rearrange(
                        "p (a c) -> p a c", a=S),
                    scale=-1.0, func=mybir.ActivationFunctionType.Exp)

    # ---- v matmuls -> v phase planes ----
    for b in range(BPC):
        for chn in range(2):
            for pr in range(2):
                for pc in range(2):
                    for s in range(2):
                        v_ps = ps.tile([128, 392], F32, tag="mm", bufs=2,
                                       name="v_ps")
                        nc.tensor.matmul(v_ps,
                                         wv_sb[:, 128 * chn:128 * (chn + 1)],
                                         xview(b, pr, pc, s),
                                         start=True, stop=True)
                        nc.scalar.copy(
                            out=plane_write(v_pl[b][chn], pr, pc, s),
                            in_=v_ps.rearrange("p (a c) -> p a c", a=S))

    # ---- rc[grp][tau] = cost_tap * r_grp  (full-width, all 2x packed) ----
    rcst = {}
    for b in range(BPC):
        for gi in range(3):
            rcst[(b, gi)] = rcpool.tile([3 * G, HO, PW], BF16,
                                        tag=f"rc{b}_{gi}", name=f"rc{b}_{gi}")
            for tau in range(3):
                eng = nc.gpsimd if tau == 1 else nc.vector
                eng.tensor_mul(
                    rcst[(b, gi)][32 * tau:32 * (tau + 1), :, 0:30],
                    tap_view(cost_pl[b], DIS[tau], DJS[gi], None, 30),
                    r_all[b][:, gi, :, 0:30])

    # ---- gamma matmuls (3 taus row-tiled into one 3-bank psum tile),
    #      psum->sbuf bf16 copy, products, per-group sums ----
    s_sb = {}
    for b in range(BPC):
        for gi in range(3):
            for chn in range(2):
                gam_sb = gampool.tile([128, 3, HO, PW], BF16,
                                      tag=f"gam{chn}", name=f"gam_sb{gi}{chn}")
                for s in range(2):
                    gam_ps = ps.tile([128, 3, 512], F32, tag="gam", bufs=2,
                                     name="gam_ps")
                    for tau in range(3):
                        gv = gam_ps[:, tau, :420].rearrange(
                            "p (a c) -> p a c", a=S)
                        nc.tensor.matmul(
                            gv, kmat_sb[32 * tau:32 * (tau + 1), gi, chn, :],
                            rcst[(b, gi)][32 * tau:32 * (tau + 1),
                                          S * s:S * (s + 1), 0:30],
                            start=True, stop=True,
                            tile_position=(32 * tau, 0))
                    nc.scalar.copy(
                        out=gam_sb[:, :, S * s:S * (s + 1), 0:30],
                        in_=gam_ps[:, :, :420].rearrange(
                            "p t (a c) -> p t a c", a=S))
                ptiles = []
                for tau in range(3):
                    p_sb = prod_pool.tile([128, HO, PW], BF16, tag="p",
                                          bufs=6, name=f"p{tau}")
                    nc.vector.tensor_mul(
                        p_sb[:, :, 0:30], gam_sb[:, tau, :, 0:30],
                        tap_view(v_pl[b][chn], DIS[tau], DJS[gi], None, 30))
                    ptiles.append(p_sb)
                ssum = spool.tile([128, HO, PW], BF16, tag=f"s{gi}_{chn}",
                                  bufs=2, name=f"ssum{gi}{chn}")
                nc.vector.tensor_add(ssum[:, :, 0:30], ptiles[0][:, :, 0:30],
                                     ptiles[1][:, :, 0:30])
                nc.vector.tensor_add(ssum[:, :, 0:30], ssum[:, :, 0:30],
                                     ptiles[2][:, :, 0:30])
                s_sb[(b, gi, chn)] = ssum

    # ---- combine groups (c' offsets 0/1/2), project with Wout, store ----
    opre = {}
    for b in range(BPC):
        for chn in range(2):
            o_t = opre_pool.tile([128, HO, PW], BF16, tag=f"op{chn}",
                                 name=f"opre{chn}")
            nc.vector.tensor_add(o_t[:, :, 0:28],
                                 s_sb[(b, 1, chn)][:, :, 0:28],
                                 s_sb[(b, 2, chn)][:, :, 2:30])
            nc.vector.tensor_add(o_t[:, :, 0:28], o_t[:, :, 0:28],
                                 s_sb[(b, 0, chn)][:, :, 1:29])
            opre[(b, chn)] = o_t

        for mo in range(2):
            for s in range(2):
                out_ps = ps.tile([128, 392], F32, tag="mm", bufs=2,
                                 name="out_ps")
                for kc in range(2):
                    nc.tensor.matmul(
                        out_ps.rearrange("p (a c) -> p a c", a=S),
                        woutT_sb[:, kc, 128 * mo:128 * (mo + 1)],
                        opre[(b, kc)][:, S * s:S * (s + 1), 0:28],
                        start=(kc == 0), stop=(kc == 1))
                o_final = outs_pool.tile([128, S, WO], F32)
                nc.scalar.copy(out=o_final,
                               in_=out_ps.rearrange("p (a c) -> p a c", a=S))
                nc.sync.dma_start(
                    out=out_d[b, 128 * mo:128 * (mo + 1),
                              S * s:S * (s + 1), :],
                    in_=o_final)


def _install_ntff_shim():
    """bass_utils expects antenv.axon_hooks (absent in this checkout)."""
    import sys
    import types
    try:
        from antenv.axon_hooks import get_axon_ntff_profile_hook  # noqa: F401
        return
    except ImportError:
        pass
    try:
        from trn_agent_boot.trn_boot import _ntff_profile_via_ctypes
        hook = _ntff_profile_via_ctypes("/opt/axon/libaxon_pjrt.so")
    except Exception:
        hook = None
    mod = types.ModuleType("antenv.axon_hooks")
    mod._hook = hook
    mod.get_axon_ntff_profile_hook = lambda: mod._hook
    mod.set_axon_ntff_profile_hook = lambda h: setattr(mod, "_hook", h)
    sys.modules["antenv.axon_hooks"] = mod


def _get_program():
    if "nc" not in _BUILD_CACHE:
        _BUILD_CACHE["nc"] = _build_program()
    return _BUILD_CACHE["nc"]


def kernel(x, Wk, Wv, Wout, q_param, attn_scale, rpb_table):
    import ml_dtypes
    x = np.ascontiguousarray(np.asarray(x, dtype=np.float32)
                             .astype(ml_dtypes.bfloat16))
    wts = _host_weights(np.asarray(Wk), np.asarray(Wv), np.asarray(Wout),
                        np.asarray(q_param), np.asarray(attn_scale),
                        np.asarray(rpb_table))
    nc = _get_program()

    in_maps = []
    for c in range(NCORES):
        in_maps.append({
            "x": np.ascontiguousarray(x[c * BPC:(c + 1) * BPC]),
            "wkq": wts["wkq"], "wv": wts["wv"], "denk": wts["denk"],
            "kmat": wts["kmat"], "woutT": wts["woutT"],
        })

    trace = bool(int(os.environ.get("KERNEL_TRACE", "0")))
    if trace:
        _install_ntff_shim()
    res = run_bass_kernel_spmd(nc, in_maps, core_ids=list(range(NCORES)),
                               trace=trace)
    _BUILD_CACHE["last_results"] = res

    out = np.empty((B, CS, HO, WO), np.float32)
    for c in range(NCORES):
        out[c * BPC:(c + 1) * BPC] = res.results[c]["out"]
    return out
